# revision 57
# baseline (speedup 1.0000x reference)
"""CrossTransformer Trainium2 kernel.

Shapes (hardcoded): B=4, TQ=TK=1024, D=1024, H=16, DK=DV=64.
Sharding: 8 cores = 4 batches x 2 query-row halves. Each core computes
out[b, qs:qs+512, :] independently (k/v work duplicated across the pair
of cores sharing a batch; no collectives).

Weights, x and y are converted to bf16 on the host.

Layout strategy: all LN affine (gamma/beta) + ReLU are applied on the
Act engine during the PSUM->SBUF copy after the PE transpose, where the
per-feature gamma/beta become per-partition scalars. Softmax exp runs on
paired-head [128,2,512] PSUM tiles; emission of attention head-pairs is
interleaved with the v-projection and attn@v chunks so the PE never
starves while the Act engine streams the exps. Softmax denominators are
broadcast across partitions on the (otherwise idle) GPSIMD engine.
"""
import sys

for _p in ("/root/.axon_site", "/root/.axon_site/_ro/trn_rl_repo",
           "/root/.axon_site/_ro/pypackages", "/opt/trn_rl_repo"):
    if _p not in sys.path:
        sys.path.append(_p)

import numpy as np
import ml_dtypes
import concourse.bass as bass
from concourse import bacc
import concourse.tile as tile
import concourse.mybir as mybir
from concourse.masks import make_identity
from concourse.bass_utils import run_bass_kernel_spmd

F32 = mybir.dt.float32
BF = mybir.dt.bfloat16
F8 = mybir.dt.float8e4
DR = mybir.MatmulPerfMode.DoubleRow
AF = mybir.ActivationFunctionType
OP = mybir.AluOpType

B, TQ, TK, D = 4, 1024, 1024, 1024
H, DK, DV = 16, 64, 64
TQC = TQ // 2          # 512 query rows per core
NT = TQC // 128        # 4 q-row tiles
KD = D // 128          # 8 contraction chunks
MASK_NEG = -30000.0

WEIGHT_NAMES = ["q_w", "k_w", "v_w", "o_w", "l1_w", "l2_w"]
FP8_WEIGHTS = {"q_w", "k_w", "v_w", "o_w"}   # l1/l2 stay bf16 (error budget)
VEC_NAMES = ["q_b", "k_b", "v_b", "o_b", "l1_b", "l2_b",
             "ln1_g", "ln1_b", "ln2_g", "ln2_b",
             "mln1_g", "mln1_b", "mln2_g", "mln2_b"]
# vectors loaded p-major [128, 8] so element d lands on partition d%128,
# column d//128 (per-partition scalars for Act in feature-major layout)
PMAJ_NAMES = ["q_b", "k_b", "ln1_g", "ln1_b", "ln2_g", "ln2_b",
              "mln1_g", "mln1_b", "mln2_g", "mln2_b"]
# vectors broadcast to [128, 1024] (free-dim biases in token-major layout)
BCAST_NAMES = ["v_b", "o_b", "l1_b", "l2_b"]


def build_kernel(compile=True, repeat=1, stop_after=None, debug_dumps=False):
    nc = bacc.Bacc()
    xs = nc.dram_tensor("xs", (TQC, D), BF, kind="ExternalInput")
    y = nc.dram_tensor("y", (TK, D), BF, kind="ExternalInput")
    mb = nc.dram_tensor("mb", (TK,), F32, kind="ExternalInput")
    wd = {n: nc.dram_tensor(n, (D, D), F8 if n in FP8_WEIGHTS else BF,
                            kind="ExternalInput") for n in WEIGHT_NAMES}
    vd = {n: nc.dram_tensor(n, (D,), F32, kind="ExternalInput") for n in VEC_NAMES}
    out = nc.dram_tensor("out", (TQC, D), F32, kind="ExternalOutput")

    dbg = {}
    if debug_dumps:
        for nm, shape, dt_ in [
            ("d_x1T", (128, KD, TQC), F8), ("d_y1T", (128, KD, TK), F8),
            ("d_qT", (128, KD, TQC), BF), ("d_kT", (128, KD, TK), BF),
            ("d_vx", (128, KD, H, DV + 1), F8), ("d_e0", (128, 2, KD, TQC), F8),
            ("d_rb0", (64, TQC), F32), ("d_attnT", (128, KD, TQC), F8),
            ("d_xsb", (128, NT, D), BF), ("d_hsb", (128, NT, D), BF),
        ]:
            dbg[nm] = nc.dram_tensor(nm, shape, dt_, kind="ExternalOutput")

    with tile.TileContext(nc) as tc:
        for r in range(repeat):
            _emit(nc, tc, xs, y, mb, wd, vd, out, pfx=f"r{r}_", stop_after=stop_after,
                  dbg=dbg if r == 0 else {})
    if compile:
        nc.compile()
    return nc


def _emit(nc, tc, xs, y, mb, wd, vd, out, pfx="", stop_after=None, dbg=None):
    dbg = dbg or {}

    def dump(nm, src):
        if nm in dbg:
            nc.sync.dma_start(dbg[nm][:], src)
    from contextlib import ExitStack

    ctx = ExitStack()
    with ctx:
        persist = ctx.enter_context(tc.tile_pool(name=pfx + "persist", bufs=1))
        lnp = ctx.enter_context(tc.tile_pool(name=pfx + "lnp", bufs=6))
        bcast = ctx.enter_context(tc.tile_pool(name=pfx + "bcast", bufs=2))
        wts = ctx.enter_context(tc.tile_pool(name=pfx + "wts", bufs=2))
        ubuf = ctx.enter_context(tc.tile_pool(name=pfx + "ubuf", bufs=2))
        xtp = ctx.enter_context(tc.tile_pool(name=pfx + "xtp", bufs=1))

        # ---------------- bulk input DMAs first (small strided const
        # loads would otherwise head-block the queue for ~10us) ----------
        x_sb = persist.tile([128, NT, D], BF, tag="x_sb", name=pfx + "x_sb")
        for t in range(NT):
            nc.sync.dma_start(x_sb[:, t, :],
                              xs.rearrange("(t p) d -> p t d", p=128)[:, t, :])
        yld_cm = tc.tile_pool(name=pfx + "yld", bufs=2)
        yld = yld_cm.__enter__()
        yls = []
        for hh in range(2):
            yl = yld.tile([128, NT, D], BF, tag="yl", name=pfx + f"yl{hh}")
            for t in range(NT):
                nc.sync.dma_start(
                    yl[:, t, :],
                    y.rearrange("(t p) d -> p t d", p=128)[:, 4 * hh + t, :])
            yls.append(yl)

        # ---------------- constants ----------------
        ident = persist.tile([128, 128], BF, tag="ident", name=pfx + "ident")
        make_identity(nc, ident)
        eps_t = persist.tile([128, 1], F32, tag="eps", name=pfx + "eps")
        nc.vector.memset(eps_t[:], 1e-5)
        ones_r = persist.tile([1, 128], BF, tag="ones_r", name=pfx + "ones_r")
        nc.vector.memset(ones_r[:], 1.0)
        # preload the Exp activation table so the first softmax exp isn't
        # delayed by an implicit table switch
        scr = persist.tile([128, 1], F32, tag="scr", name=pfx + "scr")
        nc.scalar.activation(scr[:], eps_t[:], AF.Exp, scale=1.0)
        mb_sb = persist.tile([128, KD], F32, tag="mb_sb", name=pfx + "mb_sb")
        nc.sync.dma_start(mb_sb[:], mb.rearrange("(t p) -> p t", p=128))
        pm = {}
        for n in PMAJ_NAMES:
            t = persist.tile([128, KD], F32, tag=f"pm_{n}", name=pfx + f"pm_{n}")
            nc.sync.dma_start(t[:], vd[n].rearrange("(t p) -> p t", p=128))
            pm[n] = t

        def bcast_tile(name):
            t = bcast.tile([128, D], F32, tag="bc", name=pfx + f"bc_{name}")
            nc.sync.dma_start(t[:], vd[name][:].unsqueeze(0).partition_broadcast(128))
            return t

        def load_weight(name):
            fp8 = name in FP8_WEIGHTS
            wt = wts.tile([128, KD, D], F8 if fp8 else BF,
                          tag="w8" if fp8 else "w16", name=pfx + f"w_{name}",
                          bufs=3 if fp8 else 2)
            nc.sync.dma_start(wt[:], wd[name].rearrange("(ko p) n -> p ko n", p=128))
            return wt

        # LN stats for one [128, 1024] tile -> (mv [128,2], rstd [128,1])
        def ln_stats(src, key):
            stats = lnp.tile([128, 2, 6], F32, tag="stats", name=pfx + f"st_{key}")
            for i in range(2):
                nc.vector.bn_stats(stats[:, i, :], src[:, i * 512:(i + 1) * 512])
            mv = lnp.tile([128, 2], F32, tag="mv", name=pfx + f"mv_{key}")
            nc.vector.bn_aggr(mv[:], stats[:])
            std = lnp.tile([128, 1], F32, tag="std", name=pfx + f"sd_{key}")
            nc.scalar.activation(std[:], mv[:, 1:2], AF.Sqrt, bias=eps_t[:], scale=1.0)
            rstd = lnp.tile([128, 1], F32, tag="rstd", name=pfx + f"rs_{key}")
            nc.vector.reciprocal(rstd[:], std[:])
            return mv, rstd

        # Normalize+transpose+affine+relu a token-major [128, nt, 1024]
        # source into feature-major [128, KD, nt*128] bf16 dst.
        # Transposes are emitted t-major within dt-halves so the PE can start
        # as soon as the first u chunk lands.
        def ln_t(src, nt, g, b, dst, dst_col0, psT, key, stage=None):
            u = ubuf.tile([128, nt, D], BF, tag="u", name=pfx + f"u_{key}")
            for t in range(nt):
                mv, rstd = ln_stats(src[:, t, :], f"{key}{t}")
                nc.vector.tensor_scalar(u[:, t, :], src[:, t, :], mv[:, 0:1],
                                        rstd[:], OP.subtract, OP.mult)
            for half in range(2):
                pts = [psT.tile([128, nt * 128], BF, tag="tr",
                                name=pfx + f"tr_{key}{half}_{dt}")
                       for dt in range(4)]
                for t in range(nt):
                    for dt in range(4):
                        nc.tensor.transpose(
                            pts[dt][:, t * 128:(t + 1) * 128],
                            u[:, t, (4 * half + dt) * 128:(4 * half + dt + 1) * 128],
                            ident[:])
                for dt in range(4):
                    dd = 4 * half + dt
                    if stage is None:
                        nc.scalar.activation(
                            dst[:, dd, dst_col0:dst_col0 + nt * 128], pts[dt][:],
                            AF.Relu, bias=b[:, dd:dd + 1], scale=g[:, dd:dd + 1])
                    else:
                        # park the transposed (pre-affine) rows in SBUF; the
                        # relu-affine Act is emitted later by the caller
                        nc.vector.tensor_copy(stage[:, dd, :], pts[dt][:])

        qT = persist.tile([128, KD, TQC], BF, tag="qT", name=pfx + "qT")
        kT = persist.tile([128, KD, TK], BF, tag="kT", name=pfx + "kT")
        v_ext = persist.tile([128, KD, H, DV + 1], F8, tag="v_ext", name=pfx + "v_ext")
        attnT = persist.tile([128, KD, TQC], F8, tag="attnT", name=pfx + "attnT")
        x1T = xtp.tile([128, KD, TQC], F8, tag="xT8", name=pfx + "x1T")
        y1T = persist.tile([128, KD, TK], F8, tag="yT", name=pfx + "y1T")

        # ---------------- phase A: LN + relu + transpose ----------------
        with tc.tile_pool(name=pfx + "psT", bufs=4, space="PSUM") as psT:
            ln_t(x_sb, NT, pm["ln1_g"], pm["ln1_b"], x1T, 0, psT, "x")

            yh1 = ubuf.tile([128, KD, 512], BF, tag="yh1", name=pfx + "yh1", bufs=1)
            for hh in range(2):
                ln_t(yls[hh], NT, pm["ln2_g"], pm["ln2_b"], y1T, hh * 512, psT,
                     f"y{hh}", stage=yh1 if hh == 1 else None)
        yld_cm.__exit__(None, None, None)

        dump("d_x1T", x1T[:])
        dump("d_y1T", y1T[:])
        if stop_after == "A":
            return

        # -------- phases C+D interleaved: projections + attention --------
        with (
            tc.tile_pool(name=pfx + "psP", bufs=2, space="PSUM") as psP,
            tc.tile_pool(name=pfx + "psE", bufs=2, space="PSUM") as psE,
            tc.tile_pool(name=pfx + "psV", bufs=2, space="PSUM") as psV,
            tc.tile_pool(name=pfx + "att", bufs=4) as att,
        ):
            wq = load_weight("q_w")
            wk = load_weight("k_w")
            bv = bcast_tile("v_b")
            # full-tile memset (not just the ones column): overlapping the
            # later v writes forces write-after-write ordering, avoiding a
            # byte-granularity RMW race between GPSIMD and DVE on HW
            nc.gpsimd.memset(v_ext[:], 1.0)

            def emit_q(m):
                pq = psP.tile([128, TQC], F32, tag="ps", name=pfx + f"pq{m}")
                for kc in range(0, KD, 2):
                    nc.tensor.matmul(pq[:], wq[:, kc:kc + 2, m * 128:(m + 1) * 128],
                                     x1T[:, kc:kc + 2, :], perf_mode=DR,
                                     start=(kc == 0), stop=(kc == KD - 2))
                nc.vector.tensor_scalar(qT[:, m, :], pq[:], pm["q_b"][:, m:m + 1],
                                        None, OP.add)

            def emit_k(m, hh):
                pk = psP.tile([128, 512], F32, tag="ps", name=pfx + f"pk{m}_{hh}")
                for kc in range(0, KD, 2):
                    nc.tensor.matmul(pk[:], wk[:, kc:kc + 2, m * 128:(m + 1) * 128],
                                     y1T[:, kc:kc + 2, hh * 512:(hh + 1) * 512],
                                     perf_mode=DR,
                                     start=(kc == 0), stop=(kc == KD - 2))
                nc.vector.tensor_scalar(kT[:, m, hh * 512:(hh + 1) * 512], pk[:],
                                        pm["k_b"][:, m:m + 1], None, OP.add)

            def emit_y1_affine():
                # deferred relu-affine for y's second half (staged in SBUF so
                # its Act ops don't delay the softmax exp stream)
                for dd in range(KD):
                    nc.scalar.activation(
                        y1T[:, dd, 512:1024], yh1[:, dd, :], AF.Relu,
                        bias=pm["ln2_b"][:, dd:dd + 1],
                        scale=pm["ln2_g"][:, dd:dd + 1])

            def emit_v(m, nt2, wv):
                pv = psP.tile([128, 512], F32, tag="ps", name=pfx + f"pv{m}_{nt2}")
                for kc in range(0, KD, 2):
                    nc.tensor.matmul(pv[:], y1T[:, kc:kc + 2, m * 128:(m + 1) * 128],
                                     wv[:, kc:kc + 2, nt2 * 512:(nt2 + 1) * 512],
                                     perf_mode=DR,
                                     start=(kc == 0), stop=(kc == KD - 2))
                nc.vector.tensor_tensor(
                    v_ext[:, m, nt2 * 8:(nt2 + 1) * 8, :DV],
                    pv.rearrange("p (h v) -> p h v", v=DV),
                    bv[:, nt2 * 512:(nt2 + 1) * 512].rearrange(
                        "p (h v) -> p h v", v=DV),
                    OP.add)

            e_sbs, rcs, pavs = {}, {}, {}

            def emit_logit(j, mt):
                if mt == 0:
                    e_sbs[j] = att.tile([128, 2, KD, TQC], F8, tag="e_sb",
                                        name=pfx + f"e{j}")
                ps = psE.tile([128, 2, TQC], F32, tag="pse", name=pfx + f"s{j}_{mt}")
                nc.tensor.matmul(ps[:, 0, :], kT[0:64, j, mt * 128:(mt + 1) * 128],
                                 qT[0:64, j, :], start=True, stop=True)
                nc.tensor.matmul(ps[:, 1, :], kT[64:128, j, mt * 128:(mt + 1) * 128],
                                 qT[64:128, j, :], start=True, stop=True)
                nc.scalar.activation(e_sbs[j][:, :, mt, :], ps[:], AF.Exp,
                                     bias=mb_sb[:, mt:mt + 1], scale=0.125)

            def emit_av(j, par):
                h = 2 * j + par
                if par == 0:
                    rcs[j] = att.tile([128, 2, TQC], BF, tag="rc", name=pfx + f"rc{j}", bufs=2)
                pav = psV.tile([128, TQC], F32, tag="pav", name=pfx + f"av{h}")
                pavs[(j, par)] = pav
                for kt in range(0, KD, 2):
                    nc.tensor.matmul(pav[:DV + 1, :], v_ext[:, kt:kt + 2, h, :],
                                     e_sbs[j][:, par, kt:kt + 2, :], perf_mode=DR,
                                     start=(kt == 0), stop=(kt == KD - 2))
                with nc.allow_low_precision(reason="softmax denom recip bf16"):
                    nc.vector.reciprocal(rcs[j][0:1, par, :], pav[DV:DV + 1, :])

            def emit_norm(j):
                # both broadcasts write partition-0-based rows: the GPSIMD
                # ucode does not support partition-offset outputs
                rb = att.tile([128, 2, TQC], BF, tag="rb", name=pfx + f"rb{j}", bufs=2)
                nc.gpsimd.partition_broadcast(rb[0:64, 0, :], rcs[j][0:1, 0, :])
                nc.gpsimd.partition_broadcast(rb[0:64, 1, :], rcs[j][0:1, 1, :])
                if j == 0:
                    dump("d_e0", e_sbs[0][:])
                    dump("d_rb0", rb[0:64, 0, :])
                nc.vector.tensor_tensor(attnT[0:DV, j, :], pavs[(j, 0)][:DV, :],
                                        rb[0:64, 0, :], OP.mult)
                nc.vector.tensor_tensor(attnT[DV:128, j, :], pavs[(j, 1)][:DV, :],
                                        rb[0:64, 1, :], OP.mult)
                del pavs[(j, 0)], pavs[(j, 1)], e_sbs[j], rcs[j]

            # --- schedule: logits/exp stream starts immediately after
            # q(0)/k(0); v chunks and av/norm trail as PE fillers ---
            wv = load_weight("v_w")

            def emit_pair_av(j):
                emit_av(j, 0)
                emit_av(j, 1)
                emit_norm(j)

            # pair 0 is split around y-half-1's deferred relu-affine so the
            # exp stream starts as soon as x1T + y1T[h0] are ready
            emit_q(0)
            emit_k(0, 0)
            for mt in range(4):
                emit_logit(0, mt)
            emit_y1_affine()
            emit_k(0, 1)
            for mt in range(4, KD):
                emit_logit(0, mt)
                if mt == 5:
                    emit_v(0, 0, wv)
                if mt == 7:
                    emit_v(1, 0, wv)
            for j in range(1, KD):
                emit_q(j)
                emit_k(j, 0)
                emit_k(j, 1)
                if j >= 4:
                    # free e_sb slot j-4 before pair j's exps need it
                    emit_pair_av(j - 4)
                for mt in range(KD):
                    emit_logit(j, mt)
                    if mt == 5:
                        emit_v((2 * j) % 8, j // 4, wv)
                    if mt == 7:
                        emit_v((2 * j + 1) % 8, j // 4, wv)
            for j in range(KD - 4, KD):
                emit_pair_av(j)
            dump("d_qT", qT[:])
            dump("d_kT", kT[:])
            dump("d_vx", v_ext[:])
            dump("d_attnT", attnT[:])

        if stop_after == "D":
            return

        # ---- phases E/F/G: per-token-chunk pipelined o-proj + MLP ----
        # Each mt chunk flows o-proj -> residual -> LN -> l1 (then h -> LN
        # -> l2 -> out) independently, so the PE starts each GEMM as soon
        # as that chunk's LN lands instead of waiting for all four.
        with (
            tc.tile_pool(name=pfx + "psM", bufs=4, space="PSUM") as psM,
            tc.tile_pool(name=pfx + "psT2", bufs=4, space="PSUM") as psT2,
            tc.tile_pool(name=pfx + "brow", bufs=1) as brow,
            tc.tile_pool(name=pfx + "mlp", bufs=1) as mlp,
        ):
            wo = load_weight("o_w")
            bo_f = brow.tile([1, D], F32, tag="bo_f", name=pfx + "bo_f")
            nc.sync.dma_start(bo_f[:], vd["o_b"][:].unsqueeze(0))
            bo_bf = brow.tile([1, D], BF, tag="bo_bf", name=pfx + "bo_bf")
            nc.vector.tensor_copy(bo_bf[:], bo_f[:])
            w1 = load_weight("l1_w")
            bl1 = bcast_tile("l1_b")
            w2 = load_weight("l2_w")
            bl2_f = brow.tile([1, D], F32, tag="bl2_f", name=pfx + "bl2_f")
            nc.sync.dma_start(bl2_f[:], vd["l2_b"][:].unsqueeze(0))
            bl2_bf = brow.tile([1, D], BF, tag="bl2_bf", name=pfx + "bl2_bf")
            nc.vector.tensor_copy(bl2_bf[:], bl2_f[:])
            h_sb = mlp.tile([128, NT, D], BF, tag="h_sb", name=pfx + "h_sb")
            z1T = xtp.tile([128, KD, TQC], BF, tag="xT", name=pfx + "z1T")
            z2T = mlp.tile([128, KD, TQC], BF, tag="z2T", name=pfx + "z2T")

            # LN one token chunk of src into dst[:, :, t*128:(t+1)*128]
            def ln_col(src_t, t, g, b, dst, key):
                mv, rstd = ln_stats(src_t, f"{key}{t}")
                u = ubuf.tile([128, D], BF, tag="uc", name=pfx + f"u_{key}{t}")
                nc.vector.tensor_scalar(u[:], src_t, mv[:, 0:1],
                                        rstd[:], OP.subtract, OP.mult)
                pt = psT2.tile([128, KD, 128], BF, tag="tr", name=pfx + f"tr_{key}{t}")
                for dt in range(KD):
                    nc.tensor.transpose(pt[:, dt, :],
                                        u[:, dt * 128:(dt + 1) * 128], ident[:])
                for dt in range(KD):
                    nc.scalar.activation(
                        dst[:, dt, t * 128:(t + 1) * 128], pt[:, dt, :], AF.Relu,
                        bias=b[:, dt:dt + 1], scale=g[:, dt:dt + 1])

            def emit_o_wave(mts):
                # kc-pair-major across a wave of 4 PSUM tiles: the early kc
                # chunks of attnT are ready while the last attention pairs
                # are still draining, so the PE starts o-proj ~7us sooner
                tiles = {}
                for mt in mts:
                    for nt2 in range(2):
                        tiles[(mt, nt2)] = psM.tile([128, 512], F32, tag="ps",
                                                    name=pfx + f"po{mt}_{nt2}")
                for kc in range(0, KD, 2):
                    for (mt, nt2), po in tiles.items():
                        nc.tensor.matmul(po[:],
                                         attnT[:, kc:kc + 2, mt * 128:(mt + 1) * 128],
                                         wo[:, kc:kc + 2, nt2 * 512:(nt2 + 1) * 512],
                                         perf_mode=DR,
                                         start=(kc == 0), stop=False)
                for (mt, nt2), po in tiles.items():
                    sl = slice(nt2 * 512, (nt2 + 1) * 512)
                    # rank-1 ones x o_b adds the output bias inside the GEMM
                    nc.tensor.matmul(po[:], ones_r[:], bo_bf[0:1, sl],
                                     start=False, stop=True)
                    nc.vector.tensor_tensor(x_sb[:, mt, sl], x_sb[:, mt, sl],
                                            po[:], OP.add)

            def emit_l1(mt):
                for nt2 in range(2):
                    ph = psM.tile([128, 512], F32, tag="ps",
                                  name=pfx + f"ph{mt}_{nt2}")
                    for kc in range(KD):
                        nc.tensor.matmul(ph[:], z1T[:, kc, mt * 128:(mt + 1) * 128],
                                         w1[:, kc, nt2 * 512:(nt2 + 1) * 512],
                                         start=(kc == 0), stop=(kc == KD - 1))
                    sl = slice(nt2 * 512, (nt2 + 1) * 512)
                    nc.vector.tensor_tensor(h_sb[:, mt, sl], ph[:], bl1[:, sl],
                                            OP.add)

            def emit_l2(mt, out_r):
                for nt2 in range(2):
                    pf = psM.tile([128, 512], F32, tag="ps",
                                  name=pfx + f"pf{mt}_{nt2}")
                    sl = slice(nt2 * 512, (nt2 + 1) * 512)
                    for kc in range(KD):
                        nc.tensor.matmul(pf[:], z2T[:, kc, mt * 128:(mt + 1) * 128],
                                         w2[:, kc, nt2 * 512:(nt2 + 1) * 512],
                                         start=(kc == 0), stop=False)
                    nc.tensor.matmul(pf[:], ones_r[:], bl2_bf[0:1, sl],
                                     start=False, stop=True)
                    o_sb = mlp.tile([128, 512], F32, tag="o_sb",
                                    name=pfx + f"os{mt}_{nt2}", bufs=3)
                    nc.vector.tensor_copy(o_sb[:], pf[:])
                    nc.sync.dma_start(out_r[:, mt, sl], o_sb[:])

            # software-pipelined: each consumer GEMM lags its chunk's LN by
            # one iteration so the in-order PE stream never head-blocks
            emit_o_wave([0, 1])
            ln_col(x_sb[:, 0, :], 0, pm["mln1_g"], pm["mln1_b"], z1T, "z1")
            ln_col(x_sb[:, 1, :], 1, pm["mln1_g"], pm["mln1_b"], z1T, "z1")
            emit_o_wave([2, 3])
            emit_l1(0)
            ln_col(x_sb[:, 2, :], 2, pm["mln1_g"], pm["mln1_b"], z1T, "z1")
            emit_l1(1)
            ln_col(x_sb[:, 3, :], 3, pm["mln1_g"], pm["mln1_b"], z1T, "z1")
            emit_l1(2)
            emit_l1(NT - 1)

            dump("d_xsb", x_sb[:])
            if stop_after == "E":
                return
            dump("d_hsb", h_sb[:])

            out_r = out.rearrange("(t p) d -> p t d", p=128)
            for mt in range(NT):
                ln_col(h_sb[:, mt, :], mt, pm["mln2_g"], pm["mln2_b"], z2T, "z2")
                if mt >= 1:
                    emit_l2(mt - 1, out_r)
            emit_l2(NT - 1, out_r)


_NC_CACHE = None


def _get_nc():
    global _NC_CACHE
    if _NC_CACHE is None:
        _NC_CACHE = build_kernel()
    return _NC_CACHE


def make_in_maps(inputs):
    """Split full inputs into 8 per-core input maps."""
    x = np.asarray(inputs["x"], np.float32)
    y = np.asarray(inputs["y"], np.float32)
    mask = np.asarray(inputs["mask"])
    shared = {}
    for n in WEIGHT_NAMES:
        # float8_e4m3 matches mybir.dt.np(float8e4); bit-identical to
        # e4m3fn for these small weights
        dt_ = ml_dtypes.float8_e4m3 if n in FP8_WEIGHTS else ml_dtypes.bfloat16
        shared[n] = np.ascontiguousarray(np.asarray(inputs[n], np.float32).astype(dt_))
    for n in VEC_NAMES:
        shared[n] = np.ascontiguousarray(np.asarray(inputs[n], np.float32))
    ybf = y.astype(ml_dtypes.bfloat16)
    xbf = x.astype(ml_dtypes.bfloat16)
    in_maps = []
    for c in range(8):
        b, qh = c // 2, c % 2
        m = dict(shared)
        m["xs"] = np.ascontiguousarray(xbf[b, qh * TQC:(qh + 1) * TQC, :])
        m["y"] = np.ascontiguousarray(ybf[b])
        m["mb"] = ((mask[b].astype(np.float32) - 1.0) * -MASK_NEG).astype(np.float32)
        in_maps.append(m)
    return in_maps


def assemble(results):
    outf = np.empty((B, TQ, D), np.float32)
    for c in range(8):
        b, qh = c // 2, c % 2
        outf[b, qh * TQC:(qh + 1) * TQC, :] = results[c]["out"]
    return outf


def kernel(**inputs) -> np.ndarray:
    nc = _get_nc()
    in_maps = make_in_maps(inputs)
    res = run_bass_kernel_spmd(nc, in_maps, list(range(8)))
    return assemble(res.results)


if __name__ == "__main__":
    nc = _get_nc()
    print("kernel built and compiled OK")
